# revision 14
# baseline (speedup 1.0000x reference)
"""Masked dot-product attention (B=2,H=16,L=2048,D=128) on 8 trn2 NeuronCores.

Strategy (v6 — transpose-free, bf16 streams, ring-parallel DMA):
  - Shard batch*heads: core c handles (b=0,h=2c),(0,2c+1),(1,2c),(1,2c+1) -> 4 slots.
  - The host ships Q and K already transposed to [d, seq] layout and cast to
    bf16, V in natural [seq, d] bf16. On-device the PE does ONLY the
    essential matmuls per key tile j and 512-wide q block (no transposes):
      S^T[k,q] = kT_j^T qT   (lhsT = kT_j [d,k] bf16, rhs = qT [d,512] bf16)
      O^T[d,q] += v_j^T P^T_j (lhsT = v_j [k,d] bf16, rhs = pT_j [k,512] bf16)
  - Masking costs nothing on device: the host zeroes K/V columns at positions
    >= valid_len, so masked scores are exactly 0, exp(0)=1 contributes 0 to
    O^T (V rows are zero) and exactly +1 per masked key to the softmax
    denominator, which the host subtracts as a constant afterwards.
  - exp is fused into the PSUM->SBUF eviction on the scalar engine with
    scale=1/sqrt(D), emitting bf16; up to THREE key tiles share one
    activation (st tiles span 3 PSUM banks; 2 in flight + 2 O^T accumulators
    exactly fill the 8 banks). The (block, group) units are flattened and the
    S-groups run two units ahead of the exp/PV so the PE never stalls.
  - Denominator: the DVE sums the bf16 pT tiles of a block into the block's
    column of a per-slot [128,4,512] accumulator; GpSimd folds it to 64
    partitions; the host does the final 64-way fold in numpy.
  - O^T is evicted PSUM->SBUF by the DVE (cast to bf16) and DMA'd out; the
    host transposes back to [q,d], upcasts and divides by l.
  - Every DMA ring sustains only ~23 GB/s, so transfers are chunked into
    ~128KB dma_start instructions that ride separate rings concurrently.
    q loads are per-block; the first slot's q rides the (idle) Activation
    HWDGE queue so compute starts ~7us in; the last slot's stores are
    per-block on the Activation queue so the kernel tail stays short.
    Slots run smallest-K first to minimize the head, and stores of earlier
    slots are per-2-blocks on the sync queue.
"""

import math

import numpy as np

try:
    import concourse.bass as bass
except ImportError:  # pragma: no cover
    import sys

    sys.path.append("/opt/trn_rl_repo")
    import concourse.bass as bass

import concourse.mybir as mybir
import concourse.tile as tile
from concourse import bacc
from concourse.bass_utils import run_bass_kernel_spmd

B, H, L, D = 2, 16, 2048, 128
NCORES = 8
HPC = H // NCORES  # heads per core per batch
SLOTS = B * HPC  # bh slots per core
INV_SQRT_D = 1.0 / math.sqrt(D)
F32 = mybir.dt.float32
BF16 = mybir.dt.bfloat16
QB = 4  # q blocks
QBW = L // QB  # 512 q per block
EXPF = mybir.ActivationFunctionType.Exp

_cache: dict = {}


def _build(K0: int, K1: int):
    """Build+compile the per-core program for K0/K1 valid key tiles."""
    Ks = [K0, K0, K1, K1]
    KM = max(K0, K1)
    nc = bacc.Bacc("TRN2", target_bir_lowering=False, debug=False, num_devices=NCORES)
    qT = nc.dram_tensor("qT", [SLOTS, D, L], BF16, kind="ExternalInput")
    kT = nc.dram_tensor("kT", [SLOTS, D, KM * 128], BF16, kind="ExternalInput")
    v = nc.dram_tensor("v", [SLOTS, KM * 128, D], BF16, kind="ExternalInput")
    oT = nc.dram_tensor("oT", [SLOTS, D, L], BF16, kind="ExternalOutput")
    lout = nc.dram_tensor("lout", [SLOTS, 128, QB, QBW], BF16, kind="ExternalOutput")

    with tile.TileContext(nc) as tc:
        with (
            tc.tile_pool(name="io", bufs=2) as iop,
            tc.tile_pool(name="qp", bufs=2) as qp,
            tc.tile_pool(name="work", bufs=4) as workp,
            tc.tile_pool(name="lp", bufs=2) as lp,
            tc.tile_pool(name="op", bufs=2) as op_,
            tc.tile_pool(name="psst", bufs=2, space="PSUM") as psst,
            tc.tile_pool(name="psot", bufs=2, space="PSUM") as psot,
        ):

            def emit_loads(s, first=False):
                # ordered by first use; q per block so each chunk rides its
                # own DMA ring. The first slot's q goes on the idle
                # Activation HWDGE queue to cut the time-to-first-matmul.
                Kv = Ks[s]
                qeng = nc.scalar if first else nc.sync
                kts = iop.tile([128, KM, 128], BF16, tag="kts")
                if first:
                    # per-tile chunks ride separate DMA rings so the first
                    # S-matmul (which only needs tile 0) starts ASAP
                    for t in range(Kv):
                        nc.sync.dma_start(
                            out=kts[:, t : t + 1, :],
                            in_=kT[s, :, t * 128 : (t + 1) * 128].rearrange(
                                "d (t p) -> d t p", p=128
                            ),
                        )
                else:
                    nc.sync.dma_start(
                        out=kts[:, :Kv, :],
                        in_=kT[s, :, : Kv * 128].rearrange("d (t p) -> d t p", p=128),
                    )
                qs = qp.tile([128, QB, QBW], BF16, tag="qs")
                qeng.dma_start(out=qs[:, 0, :], in_=qT[s, :, :QBW])
                vn = iop.tile([128, KM, 128], BF16, tag="vn")
                nc.sync.dma_start(
                    out=vn[:, :Kv, :],
                    in_=v[s, : Kv * 128, :].rearrange("(t p) d -> p t d", p=128),
                )
                for qb in range(1, QB):
                    qeng.dma_start(
                        out=qs[:, qb, :],
                        in_=qT[s, :, qb * QBW : (qb + 1) * QBW],
                    )
                return kts, vn, qs

            # smallest slot first (tiny head loads), second-smallest last
            # (its stores drain while the finalize runs)
            ss = sorted(range(SLOTS), key=lambda x: Ks[x])
            order = [ss[0], ss[2], ss[3], ss[1]]
            preload = {order[0]: emit_loads(order[0], first=True)}

            for idx, s in enumerate(order):
                Kv = Ks[s]
                last_slot = idx == SLOTS - 1
                kts, vn, qs = preload.pop(s)
                if idx + 1 < SLOTS:
                    nxt = order[idx + 1]
                    preload[nxt] = emit_loads(nxt)

                # j groups: triples, then pair/single remainder. The very
                # first slot warms up with a single tile then a pair so the
                # first matmul/exp only wait on tiny loads
                groups = []
                if idx == 0:
                    groups.append((0, 1))
                    if Kv >= 3:
                        groups.append((1, 2))
                    elif Kv == 2:
                        groups.append((1, 1))
                j = sum(n for _, n in groups)
                while j < Kv:
                    n = min(3, Kv - j)
                    groups.append((j, n))
                    j += n
                G = len(groups)

                laccs = lp.tile([128, QB, QBW], BF16, tag="laccs")
                o_sb = op_.tile([128, QB, QBW], BF16, tag="o_sb")

                # flatten (block, group) units so the software pipeline spans
                # block boundaries
                units = [(qb, g) for qb in range(QB) for g in range(G)]
                U = len(units)
                sts = [None] * U
                oT_pss = [None] * QB

                def emit_s(u):
                    qb, g = units[u]
                    j0, n = groups[g]
                    st = psst.tile([128, 3, QBW], F32, tag="st")
                    for jj in range(n):
                        nc.tensor.matmul(
                            st[:, jj, :],
                            kts[:, j0 + jj, :],
                            qs[:, qb, :],
                            start=True,
                            stop=True,
                        )
                    sts[u] = st

                def emit_consume(u):
                    qb, g = units[u]
                    j0, n = groups[g]
                    st = sts[u]
                    lacc = laccs[:, qb, :]
                    if g == 0:
                        oT_pss[qb] = psot.tile(
                            [128, QBW], F32, tag="oT", name="oT_ps"
                        )
                    oT_ps = oT_pss[qb]
                    pT = workp.tile([128, 3, QBW], BF16, tag="pT")
                    nc.scalar.activation(
                        pT[:, :n, :], st[:, :n, :], EXPF, scale=INV_SQRT_D
                    )
                    for jj in range(n):
                        jf = j0 + jj
                        nc.tensor.matmul(
                            oT_ps,
                            vn[:, jf, :],
                            pT[:, jj, :],
                            start=(jf == 0),
                            stop=(jf == Kv - 1),
                        )
                    # denominator accumulation on the DVE (bf16 2x rate)
                    base = 0
                    if g == 0:
                        if n >= 2:
                            nc.vector.tensor_add(lacc, pT[:, 0, :], pT[:, 1, :])
                            base = 2
                        else:
                            nc.vector.tensor_copy(lacc, pT[:, 0, :])
                            base = 1
                    for jj in range(base, n):
                        nc.vector.tensor_add(lacc, lacc, pT[:, jj, :])
                    if g == G - 1:
                        # evict O^T (cast to bf16) on the DVE
                        nc.vector.tensor_copy(o_sb[:, qb, :], oT_ps)
                        if last_slot:
                            # per-block stores on the Activation queue; the
                            # final block goes in half-chunks on two rings so
                            # the end-of-kernel drain is as short as possible
                            nchunk = 2 if qb == QB - 1 else 1
                            hw = QBW // nchunk
                            for ch in range(nchunk):
                                lo = qb * QBW + ch * hw
                                nc.scalar.dma_start(
                                    out=oT[s, :, lo : lo + hw],
                                    in_=o_sb[:, qb, ch * hw : (ch + 1) * hw],
                                )
                                nc.scalar.dma_start(
                                    out=lout[s, :, qb, ch * hw : (ch + 1) * hw],
                                    in_=laccs[:, qb, ch * hw : (ch + 1) * hw],
                                )
                        elif qb % 2 == 1:
                            nc.sync.dma_start(
                                out=oT[s, :, (qb - 1) * QBW : (qb + 1) * QBW]
                                .rearrange("d (b w) -> d b w", b=2),
                                in_=o_sb[:, qb - 1 : qb + 1, :],
                            )
                            nc.sync.dma_start(
                                out=lout[s, :, qb - 1 : qb + 1, :],
                                in_=laccs[:, qb - 1 : qb + 1, :],
                            )

                for u in range(min(2, U)):
                    emit_s(u)
                for u in range(2, U):
                    emit_s(u)
                    emit_consume(u - 2)
                for u in range(max(0, U - 2), U):
                    emit_consume(u)
    nc.compile()
    return nc


def _get_program(K0: int, K1: int):
    key = (K0, K1)
    if key not in _cache:
        _cache[key] = _build(K0, K1)
    return _cache[key]


def _run(q, k, v, valid_lens, trace=False):
    import ml_dtypes

    BF = ml_dtypes.bfloat16
    q = np.asarray(q, dtype=np.float32)
    k = np.asarray(k, dtype=np.float32)
    v = np.asarray(v, dtype=np.float32)
    vl = np.asarray(valid_lens).astype(np.int64)
    K0 = int(max(1, -(-vl[0] // 128)))
    K1 = int(max(1, -(-vl[1] // 128)))
    KM = max(K0, K1)
    nc = _get_program(K0, K1)

    Ks = [K0, K0, K1, K1]
    bs = [0, 0, 1, 1]
    nmask = [Ks[i] * 128 - int(vl[bs[i]]) for i in range(SLOTS)]

    # zero masked key positions once for the whole tensor (shared across cores)
    kz = k[:, :, : KM * 128, :].copy()
    vz = v[:, :, : KM * 128, :].astype(BF)
    for b in range(B):
        kz[b, :, vl[b] :, :] = 0.0
        vz[b, :, vl[b] :, :] = 0.0
    # [B, H, D, KM*128] transposed keys / queries in bf16
    kzT = np.ascontiguousarray(kz.transpose(0, 1, 3, 2)).astype(BF)
    qT_full = np.ascontiguousarray(q.transpose(0, 1, 3, 2)).astype(BF)

    in_maps = []
    for c in range(NCORES):
        h0, h1 = 2 * c, 2 * c + 1
        qts = np.ascontiguousarray(
            np.stack([qT_full[0, h0], qT_full[0, h1], qT_full[1, h0], qT_full[1, h1]])
        )
        kts = np.ascontiguousarray(
            np.stack([kzT[0, h0], kzT[0, h1], kzT[1, h0], kzT[1, h1]])
        )
        vs = np.ascontiguousarray(
            np.stack([vz[0, h0], vz[0, h1], vz[1, h0], vz[1, h1]])
        )
        in_maps.append({"qT": qts, "kT": kts, "v": vs})

    try:
        res = run_bass_kernel_spmd(
            nc, in_maps, core_ids=list(range(NCORES)), trace=trace
        )
    except Exception:
        # transient device wedges (NRT_EXEC_UNIT_UNRECOVERABLE) have been
        # observed to clear on retry
        res = run_bass_kernel_spmd(
            nc, in_maps, core_ids=list(range(NCORES)), trace=trace
        )

    outp = np.empty((B, H, L, D), dtype=np.float32)
    for c in range(NCORES):
        oT_dev = res.results[c]["oT"]
        l_dev = res.results[c]["lout"]
        h0, h1 = 2 * c, 2 * c + 1
        for i, (b, h) in enumerate([(0, h0), (0, h1), (1, h0), (1, h1)]):
            l = l_dev[i].astype(np.float32).sum(axis=0).reshape(L) - nmask[i]
            outp[b, h] = oT_dev[i].astype(np.float32).T / l[:, None]
    return outp, res


def kernel(q, k, v, valid_lens):
    outp, _ = _run(q, k, v, valid_lens, trace=False)
    return outp
